# revision 69
# baseline (speedup 1.0000x reference)
"""CrossAttention kernel for 8 Trainium2 NeuronCores.

Sharding (tensor-parallel heads x data-parallel batch):
  core c -> batch b = c // 4, head-group g = c % 4 (heads 4g..4g+3).
  Each core: slice Wq/Wk/Wv columns + Wo rows for its 4 heads, compute full
  attention for those heads on its batch, produce a PARTIAL output
  y_part = attn_heads @ Wo_rows  [2048, 1024] (fp16). Host sums the 4
  partials per batch in f32 and adds bo.

Per-core kernel (fp16 attention matmuls, fp8-DoubleRow projections,
PSUM accumulation f32):
  - x_q/x_c are pre-transposed ON HOST to xT [D, S] layout (free) so the
    projections need no on-chip PE transposes at all.
  - Projections run as fp8 e4m3 DoubleRow matmuls (0.5 cyc/row, 256-deep
    contraction) on host-split hi+lo operands (x scaled by 4, W by 64 to
    keep residuals out of the fp8 subnormal range), accumulating
    hi*hi + hi*lo + lo*hi in f32 PSUM: 25% fewer PE cycles than fp16 at
    ~1.7e-3 final rel err. qT/kT [dh-pack 128, S] per head pair, V natural
    [s, 4 heads, 65] with a memset 256.0 ones column (softmax denominator,
    carrying the same x*W scale as V's data columns).
  - Scores per (pair, qb, kblock): sT[keys 128, 2*512] via two K=64
    matmuls (head pair row-packed), exp on ACT (x0.125 fused) -> eT fp16.
  - PV *flipped*: stationary = eT 128x128 slices, moving = vones [128, 65]
    -> psum acc [q 128, 65] accumulated over 16 kblocks. 65-cycle matmuls
    (fp16 full rate where f32r would be 4x penalized).
  - Normalize: per-partition 1/r via vector.reciprocal + tensor_scalar_mul
    (r = col 64 of acc). attn natural [q, 2, 64] fp16.
  - PE-transpose attn 128x128 -> stackT [dh, q], Wo projection, fp16 out.
"""

import sys

sys.path.insert(0, "/opt/trn_rl_repo")

import numpy as np

B, S, D = 2, 2048, 1024
H, DH = 16, 64
P = 128
HPC = 4          # heads per core
NPAIR = 2        # head pairs per core
KC = D // P      # 8 contraction chunks for projections
NKB = S // P     # 16 key blocks of 128
NQB = 4          # q blocks of 512
QW = S // NQB    # 512
NG = 4           # context groups of 512
HD_C = HPC * DH  # 256 head dims per core
SX, SW = 4.0, 64.0  # fp8 pre-scales for x and the QKV weights

_CACHE = {}


def _build():
    from concourse import bacc, tile
    import concourse.mybir as mybir

    F16 = mybir.dt.float16
    F32 = mybir.dt.float32
    EXP = mybir.ActivationFunctionType.Exp

    nc = bacc.Bacc("TRN2", target_bir_lowering=False, debug=False)

    F8 = mybir.dt.float8e4
    DR = mybir.MatmulPerfMode.DoubleRow
    # x and the QKV weights arrive as scaled fp8 hi+lo pairs (see
    # _make_in_maps): projections run as DoubleRow fp8 matmuls (0.5 cyc/row,
    # 256-deep contraction per pass) accumulating hi*hi + hi*lo + lo*hi.
    xtq_d = nc.dram_tensor("xtq", [NQB * P, 2 * KC * QW], F8, kind="ExternalInput")
    xtc_d = nc.dram_tensor("xtc", [NG * P, 2 * KC * QW], F8, kind="ExternalInput")
    # wq/wk are stored pair-major so each head-pair's half loads contiguously
    wq_d = nc.dram_tensor("wq", [NPAIR * P, 2 * KC * P], F8, kind="ExternalInput")
    wk_d = nc.dram_tensor("wk", [NPAIR * P, 2 * KC * P], F8, kind="ExternalInput")
    wv_d = nc.dram_tensor("wv", [P, 2 * KC * HD_C], F8, kind="ExternalInput")
    wo_d = nc.dram_tensor("wo", [P, 2 * D], F16, kind="ExternalInput")
    ident_d = nc.dram_tensor("identity", [P, P], F16, kind="ExternalInput")
    y = nc.dram_tensor("y", [S, D], F16, kind="ExternalOutput")

    with tile.TileContext(nc) as tc:
        with tc.tile_pool(name="consts", bufs=1) as consts, \
             tc.tile_pool(name="wpool", bufs=1) as wpool, \
             tc.tile_pool(name="pers", bufs=1) as pers, \
             tc.tile_pool(name="xcp", bufs=4) as xcp, \
             tc.tile_pool(name="xqp", bufs=2) as xqp, \
             tc.tile_pool(name="ep", bufs=28) as ep, \
             tc.tile_pool(name="anp", bufs=4) as anp, \
             tc.tile_pool(name="skp", bufs=2) as skp, \
             tc.tile_pool(name="yp", bufs=3) as yp, \
             tc.tile_pool(name="rp", bufs=4) as rp, \
             tc.tile_pool(name="pp", bufs=2, space="PSUM") as pp, \
             tc.tile_pool(name="ps", bufs=2, space="PSUM") as ps, \
             tc.tile_pool(name="pa", bufs=2, space="PSUM") as pa:

            ident = consts.tile([P, P], F16)
            wq_sb = [wpool.tile([P, 2, KC, P], F8, name=f"wq{m}") for m in range(NPAIR)]
            wk_sb = [wpool.tile([P, 2, KC, P], F8, name=f"wk{m}") for m in range(NPAIR)]
            wv_sb = wpool.tile([P, 2, KC, HD_C], F8)
            wo_sb = wpool.tile([P, 2, D], F16)
            kT = [pers.tile([P, S], F16, name=f"kT{m}") for m in range(NPAIR)]
            qT = [pers.tile([P, S], F16, name=f"qT{m}") for m in range(NPAIR)]
            # V for all 4 heads: [s-in-block, kblock, head, dh+1]
            vones = pers.tile([P, NKB, HPC, DH + 1], F16)
            # ones column scaled by SX*SW (x and W arrive pre-scaled; the
            # denominator column must carry the same scale as V's data cols)
            nc.gpsimd.memset(vones[:, :, :, DH:DH + 1], 256.0)

            def load_w(sb, d, m):
                nc.sync.dma_start(
                    out=sb[m],
                    in_=d.ap()[m * P:(m + 1) * P, :].rearrange(
                        "p (l c f) -> p l c f", l=2, f=P))

            xc_t, xq_t, pa_t, st_t = {}, {}, {}, {}

            def load_late_consts():
                nc.sync.dma_start(
                    out=wo_sb, in_=wo_d.ap().rearrange("p (a f) -> p a f", f=D))
                nc.sync.dma_start(out=ident, in_=ident_d.ap())

            def load_xc(g, halves=1):
                t = xcp.tile([P, 2, KC, QW], F8, tag="xc", name=f"xc{g}")
                src = xtc_d.ap()[g * P:(g + 1) * P, :].rearrange(
                    "p (l c s) -> p l c s", l=2, s=QW)
                hc = KC // halves
                for h in range(halves):
                    nc.sync.dma_start(
                        out=t[:, :, h * hc:(h + 1) * hc, :],
                        in_=src[:, :, h * hc:(h + 1) * hc, :])
                xc_t[g] = t

            def load_xq(qb, halves=1):
                t = xqp.tile([P, 2, KC, QW], F8, tag="xq", name=f"xq{qb}")
                src = xtq_d.ap()[qb * P:(qb + 1) * P, :].rearrange(
                    "p (l c s) -> p l c s", l=2, s=QW)
                hc = KC // halves
                for h in range(halves):
                    nc.sync.dma_start(
                        out=t[:, :, h * hc:(h + 1) * hc, :],
                        in_=src[:, :, h * hc:(h + 1) * hc, :])
                xq_t[qb] = t

            kh_t, qh_t = {}, {}

            def k_proj(g, m):
                k_half(g, m, 0)
                k_half(g, m, 1)

            # hi*hi + hi*lo + lo*hi accumulation terms: (x half, w half)
            HL = ((0, 0), (0, 1), (1, 0))

            def k_half(g, m, h):
                """Half of a K projection (2 DoubleRow chunk-pairs x 3 hi/lo
                terms): split so a full projection never blocks the next
                scores in the in-order PE queue longer than one exp shadow."""
                xt = xc_t[g]
                if h == 0:
                    kh_t[(g, m)] = pp.tile(
                        [P, QW], F32, tag="pp", name=f"kps{g}_{m}")
                kps = kh_t[(g, m)]
                for t in range(2 * h, 2 * h + 2):
                    for j, (xl, wl) in enumerate(HL):
                        nc.tensor.matmul(
                            kps[:], wk_sb[m][:, wl, 2 * t:2 * t + 2, :],
                            xt[:, xl, 2 * t:2 * t + 2, :],
                            start=(t == 0 and j == 0),
                            stop=(t == 3 and j == len(HL) - 1),
                            perf_mode=DR)
                if h == 1:
                    nc.vector.tensor_copy(
                        out=kT[m][:, g * QW:(g + 1) * QW], in_=kps[:])
                    del kh_t[(g, m)]

            def v_proj(blk):
                g, sb = blk // 4, blk % 4
                xt = xc_t[g]
                vps = pp.tile([P, HD_C], F32, tag="pp", name=f"vps{blk}")
                for t in range(4):
                    for j, (xl, wl) in enumerate(HL):
                        nc.tensor.matmul(
                            vps[:],
                            xt[:, xl, 2 * t:2 * t + 2, sb * P:(sb + 1) * P],
                            wv_sb[:, wl, 2 * t:2 * t + 2, :],
                            start=(t == 0 and j == 0),
                            stop=(t == 3 and j == len(HL) - 1),
                            perf_mode=DR)
                nc.vector.tensor_copy(
                    out=vones[:, blk, :, 0:DH],
                    in_=vps[:].rearrange("p (h d) -> p h d", h=HPC))

            def qpath(qb, pairs=(0, 1)):
                for m in pairs:
                    q_half(qb, m, 0)
                    q_half(qb, m, 1)

            def q_half(qb, m, h):
                xt = xq_t[qb]
                if h == 0:
                    qh_t[(qb, m)] = pp.tile(
                        [P, QW], F32, tag="pp", name=f"qps{qb}_{m}")
                qps = qh_t[(qb, m)]
                for t in range(2 * h, 2 * h + 2):
                    for j, (xl, wl) in enumerate(HL):
                        nc.tensor.matmul(
                            qps[:], wq_sb[m][:, wl, 2 * t:2 * t + 2, :],
                            xt[:, xl, 2 * t:2 * t + 2, :],
                            start=(t == 0 and j == 0),
                            stop=(t == 3 and j == len(HL) - 1),
                            perf_mode=DR)
                if h == 1:
                    nc.vector.tensor_copy(
                        out=qT[m][:, qb * QW:(qb + 1) * QW], in_=qps[:])
                    del qh_t[(qb, m)]

            eT_t, pvq = {}, {}

            def se(qb, m, i):
                """Scores + exp for kblock i of (qb, pair m); queue its PV."""
                sT = ps.tile([P, 2, QW], F32, tag="s", name=f"sT{qb}_{m}_{i}")
                for hh in range(2):
                    nc.tensor.matmul(
                        sT[:, hh, :],
                        kT[m][hh * DH:(hh + 1) * DH, i * P:(i + 1) * P],
                        qT[m][hh * DH:(hh + 1) * DH, qb * QW:(qb + 1) * QW],
                        start=True, stop=True)
                eT = ep.tile([P, 2, QW], F16, tag="e", name=f"eT{qb}_{m}_{i}")
                nc.scalar.activation(
                    out=eT[:], in_=sT[:], func=EXP,
                    scale=float(DH) ** -0.5 / (SX * SW) ** 2)
                eT_t[(qb, m, i)] = eT
                pvq.setdefault((qb, m), []).append(i)

            def pv(qb, m, i):
                """PV accumulate for kblock i. pa: start=True zeroes the whole
                2KB PSUM bank (pending-zero is bank-granular), so only the
                first region (qs==0) asserts it; qs 1-3's first writes land on
                pending-zero bytes and overwrite rather than accumulate."""
                if (qb, m) not in pa_t:
                    pa_t[(qb, m)] = [
                        pa.tile([P, 4, P], F32, tag="pa", name=f"pa{qb}_{m}_{h}")
                        for h in range(2)
                    ]
                acc = pa_t[(qb, m)]
                eT = eT_t.pop((qb, m, i))
                for hh in range(2):
                    for qs in range(4):
                        nc.tensor.matmul(
                            acc[hh][:, qs, 0:DH + 1],
                            eT[:, hh, qs * P:(qs + 1) * P],
                            vones[:, i, 2 * m + hh, :],
                            start=(i == 0 and qs == 0), stop=(i == NKB - 1))

            def flush_pv(qb, m, n=None, keep=0):
                """Emit unit (qb, m)'s queued PVs in FIFO order: up to n of
                them (None = all), always leaving at least `keep` pending."""
                q = pvq.get((qb, m), [])
                cnt = 0
                while q and len(q) > keep and (n is None or cnt < n):
                    pv(qb, m, q.pop(0))
                    cnt += 1

            def normalize_T(qb, m, wo_qsubs=()):
                acc = pa_t[(qb, m)]
                if m == 0:
                    st_t[qb] = skp.tile([P, 2, QW], F16, tag="st", name=f"st{qb}")
                recs = []
                for hh in range(2):
                    rec = rp.tile([P, 4, 1], F32, tag="r", name=f"rec{qb}_{m}_{hh}")
                    nc.vector.reciprocal(out=rec[:], in_=acc[hh][:, :, DH:DH + 1])
                    recs.append(rec)
                for qs in range(4):
                    an = anp.tile([P, 2, DH], F16, tag="a", name=f"an{qb}_{m}_{qs}")
                    for hh in range(2):
                        nc.vector.tensor_scalar_mul(
                            out=an[:, hh, :], in0=acc[hh][:, qs, 0:DH],
                            scalar1=recs[hh][:, qs, :])
                    pt = pp.tile([P, P], F16, tag="pp", name=f"pt{qb}_{m}_{qs}")
                    nc.tensor.transpose(
                        pt[:], an[:].rearrange("p a b -> p (a b)"), ident[:])
                    nc.vector.tensor_copy(
                        out=st_t[qb][:, m, qs * P:(qs + 1) * P], in_=pt[:])
                    if qs in wo_qsubs:
                        wo_proj(qb, (qs,), drain_act=True)

            def wo_proj(qb, qsubs=(0, 1, 2, 3), drain_act=False):
                st = st_t[qb]
                for qs in qsubs:
                    ysb = yp.tile([P, D], F16, tag="y", name=f"ysb{qb}_{qs}")
                    for nb in range(2):
                        yps = pp.tile([P, 512], F32, tag="pp", name=f"yps{qb}_{qs}_{nb}")
                        for ch in range(2):
                            nc.tensor.matmul(
                                yps[:], st[:, ch, qs * P:(qs + 1) * P],
                                wo_sb[:, ch, nb * 512:(nb + 1) * 512],
                                start=(ch == 0), stop=(ch == 1))
                        # tail: ACT is idle once the last exp is done; the two
                        # halves drain on ACT and DVE in parallel, and each
                        # half DMAs out as soon as it lands
                        if drain_act and nb == 0:
                            nc.scalar.copy(
                                out=ysb[:, nb * 512:(nb + 1) * 512], in_=yps[:])
                        else:
                            nc.vector.tensor_copy(
                                out=ysb[:, nb * 512:(nb + 1) * 512], in_=yps[:])
                        if drain_act:
                            nc.sync.dma_start(
                                out=y.ap()[(qb * 4 + qs) * P:(qb * 4 + qs + 1) * P,
                                           nb * 512:(nb + 1) * 512],
                                in_=ysb[:, nb * 512:(nb + 1) * 512])
                    if not drain_act:
                        nc.sync.dma_start(
                            out=y.ap()[(qb * 4 + qs) * P:(qb * 4 + qs + 1) * P, :],
                            in_=ysb)

            # ---- emission schedule ----
            # Principle: exp on ACT is the global bottleneck (~133us busy)
            # and PE busy is nearly equal, so every phase must interleave
            # scores/exp with just enough other PE work. The warm (ctx) phase
            # is PE-heavy, so BOTH qb0 units' scores/exp streams run there
            # while unit (0,1)'s PVs are deferred (PV accumulation order is
            # free) until its pa banks free up after normalize(0,0). Dummy
            # matmuls at t=0 ramp the PE out of its low p-state before real
            # data lands. Steady units keep PVs 2 blocks behind scores so a
            # waiting PV never stalls score issue in the in-order PE queue.
            dumW = consts.tile([P, P], F16)
            dumM = consts.tile([P, QW], F16)
            nc.gpsimd.memset(dumW[:], 0.0)
            nc.gpsimd.memset(dumM[:], 0.0)
            for d in range(12):
                dmp = pp.tile([P, QW], F32, tag="pp", name=f"dum{d}")
                nc.tensor.matmul(dmp[:], dumW[:], dumM[:], start=True, stop=True)
            load_w(wk_sb, wk_d, 0)
            load_w(wq_sb, wq_d, 0)
            xc0 = xcp.tile([P, 2, KC, QW], F8, tag="xc", name="xc0")
            xq0 = xqp.tile([P, 2, KC, QW], F8, tag="xq", name="xq0")
            xc_t[0], xq_t[0] = xc0, xq0
            src_c = xtc_d.ap()[0:P, :].rearrange("p (l c s) -> p l c s", l=2, s=QW)
            src_q = xtq_d.ap()[0:P, :].rearrange("p (l c s) -> p l c s", l=2, s=QW)
            # interleave xc0/xq0 half-DMAs so the first K and Q projection
            # halves ping-pong with the (serial) DMA pool
            nc.sync.dma_start(out=xc0[:, :, 0:4, :], in_=src_c[:, :, 0:4, :])
            nc.sync.dma_start(out=xq0[:, :, 0:4, :], in_=src_q[:, :, 0:4, :])
            nc.sync.dma_start(out=xc0[:, :, 4:8, :], in_=src_c[:, :, 4:8, :])
            nc.sync.dma_start(out=xq0[:, :, 4:8, :], in_=src_q[:, :, 4:8, :])
            load_w(wk_sb, wk_d, 1)
            load_w(wq_sb, wq_d, 1)
            nc.sync.dma_start(
                out=wv_sb,
                in_=wv_d.ap().rearrange("p (l c f) -> p l c f", l=2, f=HD_C))
            load_xc(1)
            k_half(0, 0, 0)
            q_half(0, 0, 0)
            k_half(0, 0, 1)
            q_half(0, 0, 1)
            se(0, 0, 0)
            k_proj(0, 1)
            se(0, 0, 1)
            qpath(0, (1,))
            se(0, 1, 0)
            v_proj(0)
            load_late_consts()
            load_xc(2)
            load_xc(3)
            # Warm loop: stream (0,0) leads, stream (0,1) lags 2 blocks so an
            # exp is always ready; each slot carries <=1 quarter-projection +
            # one V block between scores, so no PE quantum outlasts one exp
            # shadow. (0,1)'s PVs defer entirely (pa banks busy until
            # normalize(0,0)); (0,0)'s trail by 2.
            for i in range(2, NKB):
                if i == 4:
                    load_xq(1)
                if i < 14:
                    # group g's 4 quarter-projections run at slots 4g-2..4g+1
                    q4 = (i + 2) % 4
                    k_half((i + 2) // 4, q4 // 2, q4 % 2)
                if i in (8, 9):
                    q_half(1, 0, i - 8)
                se(0, 1, i - 1)
                v_proj(i - 1)
                se(0, 0, i)
                # third stream: unit (1,0)'s first blocks ride the warm tail
                if i >= 10:
                    se(1, 0, i - 10)
                flush_pv(0, 0, n=1, keep=2)
            v_proj(15)
            se(0, 1, 15)

            # unit (1,0) remainder: drain deferred qb0 PVs in its ACT shadow
            se(1, 0, 6)
            flush_pv(0, 0)
            normalize_T(0, 0)
            se(1, 0, 7)
            flush_pv(0, 1, n=5)
            se(1, 0, 8)
            flush_pv(0, 1, n=5)
            se(1, 0, 9)
            flush_pv(0, 1, n=6)
            normalize_T(0, 1)
            for i in range(10, NKB):
                if i in (10, 11):
                    q_half(1, 1, i - 10)
                if i == 12:
                    wo_proj(0, (0,))
                if i == 14:
                    wo_proj(0, (1,))
                se(1, 0, i)
                flush_pv(1, 0, n=2, keep=2)

            for u in range(3, 2 * NQB):
                qb, m = u // 2, u % 2
                pqb, pm = (u - 1) // 2, (u - 1) % 2
                se(qb, m, 0)
                flush_pv(pqb, pm, n=6)
                se(qb, m, 1)
                flush_pv(pqb, pm)
                normalize_T(pqb, pm)
                for i in range(2, NKB):
                    if m == 0 and qb >= 2 and i in (4, 6, 8, 10):
                        wo_proj(qb - 1, ((i - 4) // 2,))
                    if m == 0 and i in (12, 13):
                        q_half(qb, 1, i - 12)
                    if u == 3 and i in (4, 6):
                        wo_proj(0, (i // 2,))  # qsubs 2, 3 of qb0
                    if m == 1 and qb + 1 < NQB:
                        if i == 6:
                            load_xq(qb + 1)
                        if i in (8, 9):
                            q_half(qb + 1, 0, i - 8)
                    se(qb, m, i)
                    flush_pv(qb, m, n=1, keep=2)
            flush_pv(NQB - 1, 1)
            normalize_T(NQB - 1, 1, wo_qsubs=(0, 1, 2, 3))

    nc.compile()
    return nc


def _get_nc():
    if "nc" not in _CACHE:
        _CACHE["nc"] = _build()
    return _CACHE["nc"]


def _hilo(a, axis):
    """Split scaled f32 array into fp8 hi + fp8 lo stacked on `axis`."""
    import ml_dtypes

    hi = a.astype(ml_dtypes.float8_e4m3fn)
    lo = (a - hi.astype(np.float32)).astype(ml_dtypes.float8_e4m3fn)
    return np.ascontiguousarray(np.stack([hi, lo], axis=axis))


def _make_in_maps(query, context, Wq, Wk, Wv, Wo):
    ident = np.eye(P, dtype=np.float16)
    in_maps = []
    for c in range(8):
        b, g = c // 4, c % 4
        csl = slice(g * HD_C, (g + 1) * HD_C)
        # xT [D, S] -> [qb, p, hilo, c, s] -> [(qb p), (l c s)], scaled by SX
        xq = _hilo(
            (SX * query[b].T).astype(np.float32)
            .reshape(KC, P, NQB, QW).transpose(2, 1, 0, 3), axis=2
        ).reshape(NQB * P, 2 * KC * QW)
        xc = _hilo(
            (SX * context[b].T).astype(np.float32)
            .reshape(KC, P, NG, QW).transpose(2, 1, 0, 3), axis=2
        ).reshape(NG * P, 2 * KC * QW)
        # Wq/Wk [D, 256] -> pair-major [m, p, hilo, c, f], scaled by SW
        wq = _hilo(
            (SW * Wq[:, csl]).astype(np.float32)
            .reshape(KC, P, NPAIR, P).transpose(2, 1, 0, 3), axis=2
        ).reshape(NPAIR * P, 2 * KC * P)
        wk = _hilo(
            (SW * Wk[:, csl]).astype(np.float32)
            .reshape(KC, P, NPAIR, P).transpose(2, 1, 0, 3), axis=2
        ).reshape(NPAIR * P, 2 * KC * P)
        # Wv [D, 256] -> [p, hilo, c, f], scaled by SW
        wv = _hilo(
            (SW * Wv[:, csl]).astype(np.float32)
            .reshape(KC, P, HD_C).transpose(1, 0, 2), axis=1
        ).reshape(P, 2 * KC * HD_C)
        # Wo rows [256, D] -> [p, (ch f)] with element [p, ch, f] = Wo[ch*128+p, f]
        wo = np.ascontiguousarray(
            Wo[csl, :].reshape(2, P, D).transpose(1, 0, 2)
            .reshape(P, 2 * D).astype(np.float16))
        in_maps.append({
            "xtq": xq,
            "xtc": xc,
            "wq": wq,
            "wk": wk,
            "wv": wv,
            "wo": wo,
            "identity": ident,
        })
    return in_maps


def kernel(query, context, Wq, Wk, Wv, Wo, bo):
    from concourse.bass_utils import run_bass_kernel_spmd

    query = np.asarray(query, dtype=np.float32)
    context = np.asarray(context, dtype=np.float32)
    Wq = np.asarray(Wq, dtype=np.float32)
    Wk = np.asarray(Wk, dtype=np.float32)
    Wv = np.asarray(Wv, dtype=np.float32)
    Wo = np.asarray(Wo, dtype=np.float32)
    bo = np.asarray(bo, dtype=np.float32)

    nc = _get_nc()
    in_maps = _make_in_maps(query, context, Wq, Wk, Wv, Wo)
    res = run_bass_kernel_spmd(nc, in_maps, core_ids=list(range(8)))
    out = np.zeros((B, S, D), np.float32)
    for c in range(8):
        out[c // 4] += np.asarray(res.results[c]["y"], dtype=np.float32)
    out += bo[None, None, :]
    return out


# revision 70
# speedup vs baseline: 1.0243x; 1.0243x over previous
"""CrossAttention kernel for 8 Trainium2 NeuronCores.

Sharding (tensor-parallel heads x data-parallel batch):
  core c -> batch b = c // 4, head-group g = c % 4 (heads 4g..4g+3).
  Each core: slice Wq/Wk/Wv columns + Wo rows for its 4 heads, compute full
  attention for those heads on its batch, produce a PARTIAL output
  y_part = attn_heads @ Wo_rows  [2048, 1024] (fp16). Host sums the 4
  partials per batch in f32 and adds bo.

Per-core kernel (fp16 attention matmuls, fp8-DoubleRow projections,
PSUM accumulation f32):
  - x_q/x_c are pre-transposed ON HOST to xT [D, S] layout (free) so the
    projections need no on-chip PE transposes at all.
  - Projections run as fp8 e4m3 DoubleRow matmuls (0.5 cyc/row, 256-deep
    contraction) on host-split hi+lo operands (x scaled by 4, W by 64 to
    keep residuals out of the fp8 subnormal range), accumulating
    hi*hi + hi*lo + lo*hi in f32 PSUM: 25% fewer PE cycles than fp16 at
    ~1.7e-3 final rel err. qT/kT [dh-pack 128, S] per head pair, V natural
    [s, 4 heads, 65] with a memset 256.0 ones column (softmax denominator,
    carrying the same x*W scale as V's data columns).
  - Scores per (pair, qb, kblock): sT[keys 128, 2*512] via two K=64
    matmuls (head pair row-packed), exp on ACT (x0.125 fused) -> eT fp16.
  - PV *flipped*: stationary = eT 128x128 slices, moving = vones [128, 65]
    -> psum acc [q 128, 65] accumulated over 16 kblocks. 65-cycle matmuls
    (fp16 full rate where f32r would be 4x penalized).
  - Normalize: per-partition 1/r via vector.reciprocal + tensor_scalar_mul
    (r = col 64 of acc). attn natural [q, 2, 64] fp16.
  - PE-transpose attn 128x128 -> stackT [dh, q], Wo projection, fp16 out.
"""

import sys

sys.path.insert(0, "/opt/trn_rl_repo")

import numpy as np

B, S, D = 2, 2048, 1024
H, DH = 16, 64
P = 128
HPC = 4          # heads per core
NPAIR = 2        # head pairs per core
KC = D // P      # 8 contraction chunks for projections
NKB = S // P     # 16 key blocks of 128
NQB = 4          # q blocks of 512
QW = S // NQB    # 512
NG = 4           # context groups of 512
HD_C = HPC * DH  # 256 head dims per core
SX, SW = 4.0, 64.0  # fp8 pre-scales for x and the QKV weights

_CACHE = {}


def _build():
    from concourse import bacc, tile
    import concourse.mybir as mybir

    F16 = mybir.dt.float16
    F32 = mybir.dt.float32
    EXP = mybir.ActivationFunctionType.Exp

    nc = bacc.Bacc("TRN2", target_bir_lowering=False, debug=False)

    F8 = mybir.dt.float8e4
    DR = mybir.MatmulPerfMode.DoubleRow
    # x and the QKV weights arrive as scaled fp8 hi+lo pairs (see
    # _make_in_maps): projections run as DoubleRow fp8 matmuls (0.5 cyc/row,
    # 256-deep contraction per pass) accumulating hi*hi + hi*lo + lo*hi.
    xtq_d = nc.dram_tensor("xtq", [NQB * P, 2 * KC * QW], F8, kind="ExternalInput")
    xtc_d = nc.dram_tensor("xtc", [NG * P, 2 * KC * QW], F8, kind="ExternalInput")
    # wq/wk are stored pair-major so each head-pair's half loads contiguously
    wq_d = nc.dram_tensor("wq", [NPAIR * P, 2 * KC * P], F8, kind="ExternalInput")
    wk_d = nc.dram_tensor("wk", [NPAIR * P, 2 * KC * P], F8, kind="ExternalInput")
    wv_d = nc.dram_tensor("wv", [P, 2 * KC * HD_C], F8, kind="ExternalInput")
    wo_d = nc.dram_tensor("wo", [P, 2 * D], F16, kind="ExternalInput")
    ident_d = nc.dram_tensor("identity", [P, P], F16, kind="ExternalInput")
    y = nc.dram_tensor("y", [S, D], F16, kind="ExternalOutput")

    with tile.TileContext(nc) as tc:
        with tc.tile_pool(name="consts", bufs=1) as consts, \
             tc.tile_pool(name="wpool", bufs=1) as wpool, \
             tc.tile_pool(name="pers", bufs=1) as pers, \
             tc.tile_pool(name="xcp", bufs=4) as xcp, \
             tc.tile_pool(name="xqp", bufs=2) as xqp, \
             tc.tile_pool(name="ep", bufs=28) as ep, \
             tc.tile_pool(name="anp", bufs=4) as anp, \
             tc.tile_pool(name="skp", bufs=2) as skp, \
             tc.tile_pool(name="yp", bufs=3) as yp, \
             tc.tile_pool(name="rp", bufs=4) as rp, \
             tc.tile_pool(name="pp", bufs=2, space="PSUM") as pp, \
             tc.tile_pool(name="ps", bufs=2, space="PSUM") as ps, \
             tc.tile_pool(name="pa", bufs=2, space="PSUM") as pa:

            ident = consts.tile([P, P], F16)
            wq_sb = [wpool.tile([P, 2, KC, P], F8, name=f"wq{m}") for m in range(NPAIR)]
            wk_sb = [wpool.tile([P, 2, KC, P], F8, name=f"wk{m}") for m in range(NPAIR)]
            wv_sb = wpool.tile([P, 2, KC, HD_C], F8)
            wo_sb = wpool.tile([P, 2, D], F16)
            kT = [pers.tile([P, S], F16, name=f"kT{m}") for m in range(NPAIR)]
            qT = [pers.tile([P, S], F16, name=f"qT{m}") for m in range(NPAIR)]
            # V for all 4 heads: [s-in-block, kblock, head, dh+1]
            vones = pers.tile([P, NKB, HPC, DH + 1], F16)
            # ones column scaled by SX*SW (x and W arrive pre-scaled; the
            # denominator column must carry the same scale as V's data cols)
            nc.gpsimd.memset(vones[:, :, :, DH:DH + 1], 256.0)

            def load_w(sb, d, m):
                nc.sync.dma_start(
                    out=sb[m],
                    in_=d.ap()[m * P:(m + 1) * P, :].rearrange(
                        "p (l c f) -> p l c f", l=2, f=P))

            xc_t, xq_t, pa_t, st_t = {}, {}, {}, {}

            def load_late_consts():
                nc.sync.dma_start(
                    out=wo_sb, in_=wo_d.ap().rearrange("p (a f) -> p a f", f=D))
                nc.sync.dma_start(out=ident, in_=ident_d.ap())

            def load_xc(g, halves=1):
                t = xcp.tile([P, 2, KC, QW], F8, tag="xc", name=f"xc{g}")
                src = xtc_d.ap()[g * P:(g + 1) * P, :].rearrange(
                    "p (l c s) -> p l c s", l=2, s=QW)
                hc = KC // halves
                for h in range(halves):
                    nc.sync.dma_start(
                        out=t[:, :, h * hc:(h + 1) * hc, :],
                        in_=src[:, :, h * hc:(h + 1) * hc, :])
                xc_t[g] = t

            def load_xq(qb, halves=1):
                t = xqp.tile([P, 2, KC, QW], F8, tag="xq", name=f"xq{qb}")
                src = xtq_d.ap()[qb * P:(qb + 1) * P, :].rearrange(
                    "p (l c s) -> p l c s", l=2, s=QW)
                hc = KC // halves
                for h in range(halves):
                    nc.sync.dma_start(
                        out=t[:, :, h * hc:(h + 1) * hc, :],
                        in_=src[:, :, h * hc:(h + 1) * hc, :])
                xq_t[qb] = t

            kh_t, qh_t = {}, {}

            def k_proj(g, m):
                k_half(g, m, 0)
                k_half(g, m, 1)

            # hi*hi + hi*lo + lo*hi accumulation terms: (x half, w half)
            HL = ((0, 0), (0, 1), (1, 0))

            def k_half(g, m, h):
                """Half of a K projection (2 DoubleRow chunk-pairs x 3 hi/lo
                terms): split so a full projection never blocks the next
                scores in the in-order PE queue longer than one exp shadow."""
                xt = xc_t[g]
                if h == 0:
                    kh_t[(g, m)] = pp.tile(
                        [P, QW], F32, tag="pp", name=f"kps{g}_{m}")
                kps = kh_t[(g, m)]
                for t in range(2 * h, 2 * h + 2):
                    for j, (xl, wl) in enumerate(HL):
                        nc.tensor.matmul(
                            kps[:], wk_sb[m][:, wl, 2 * t:2 * t + 2, :],
                            xt[:, xl, 2 * t:2 * t + 2, :],
                            start=(t == 0 and j == 0),
                            stop=(t == 3 and j == len(HL) - 1),
                            perf_mode=DR)
                if h == 1:
                    nc.vector.tensor_copy(
                        out=kT[m][:, g * QW:(g + 1) * QW], in_=kps[:])
                    del kh_t[(g, m)]

            def v_proj(blk):
                g, sb = blk // 4, blk % 4
                xt = xc_t[g]
                vps = pp.tile([P, HD_C], F32, tag="pp", name=f"vps{blk}")
                for t in range(4):
                    for j, (xl, wl) in enumerate(HL):
                        nc.tensor.matmul(
                            vps[:],
                            xt[:, xl, 2 * t:2 * t + 2, sb * P:(sb + 1) * P],
                            wv_sb[:, wl, 2 * t:2 * t + 2, :],
                            start=(t == 0 and j == 0),
                            stop=(t == 3 and j == len(HL) - 1),
                            perf_mode=DR)
                nc.vector.tensor_copy(
                    out=vones[:, blk, :, 0:DH],
                    in_=vps[:].rearrange("p (h d) -> p h d", h=HPC))

            def qpath(qb, pairs=(0, 1)):
                for m in pairs:
                    q_half(qb, m, 0)
                    q_half(qb, m, 1)

            def q_half(qb, m, h):
                xt = xq_t[qb]
                if h == 0:
                    qh_t[(qb, m)] = pp.tile(
                        [P, QW], F32, tag="pp", name=f"qps{qb}_{m}")
                qps = qh_t[(qb, m)]
                for t in range(2 * h, 2 * h + 2):
                    for j, (xl, wl) in enumerate(HL):
                        nc.tensor.matmul(
                            qps[:], wq_sb[m][:, wl, 2 * t:2 * t + 2, :],
                            xt[:, xl, 2 * t:2 * t + 2, :],
                            start=(t == 0 and j == 0),
                            stop=(t == 3 and j == len(HL) - 1),
                            perf_mode=DR)
                if h == 1:
                    nc.vector.tensor_copy(
                        out=qT[m][:, qb * QW:(qb + 1) * QW], in_=qps[:])
                    del qh_t[(qb, m)]

            eT_t, pvq = {}, {}

            def se(qb, m, i):
                """Scores + exp for kblock i of (qb, pair m); queue its PV."""
                sT = ps.tile([P, 2, QW], F32, tag="s", name=f"sT{qb}_{m}_{i}")
                for hh in range(2):
                    nc.tensor.matmul(
                        sT[:, hh, :],
                        kT[m][hh * DH:(hh + 1) * DH, i * P:(i + 1) * P],
                        qT[m][hh * DH:(hh + 1) * DH, qb * QW:(qb + 1) * QW],
                        start=True, stop=True)
                eT = ep.tile([P, 2, QW], F16, tag="e", name=f"eT{qb}_{m}_{i}")
                nc.scalar.activation(
                    out=eT[:], in_=sT[:], func=EXP,
                    scale=float(DH) ** -0.5 / (SX * SW) ** 2)
                eT_t[(qb, m, i)] = eT
                pvq.setdefault((qb, m), []).append(i)

            def pv(qb, m, i):
                """PV accumulate for kblock i. pa: start=True zeroes the whole
                2KB PSUM bank (pending-zero is bank-granular), so only the
                first region (qs==0) asserts it; qs 1-3's first writes land on
                pending-zero bytes and overwrite rather than accumulate."""
                if (qb, m) not in pa_t:
                    pa_t[(qb, m)] = [
                        pa.tile([P, 4, P], F32, tag="pa", name=f"pa{qb}_{m}_{h}")
                        for h in range(2)
                    ]
                acc = pa_t[(qb, m)]
                eT = eT_t.pop((qb, m, i))
                for hh in range(2):
                    for qs in range(4):
                        nc.tensor.matmul(
                            acc[hh][:, qs, 0:DH + 1],
                            eT[:, hh, qs * P:(qs + 1) * P],
                            vones[:, i, 2 * m + hh, :],
                            start=(i == 0 and qs == 0), stop=(i == NKB - 1))

            def flush_pv(qb, m, n=None, keep=0):
                """Emit unit (qb, m)'s queued PVs in FIFO order: up to n of
                them (None = all), always leaving at least `keep` pending."""
                q = pvq.get((qb, m), [])
                cnt = 0
                while q and len(q) > keep and (n is None or cnt < n):
                    pv(qb, m, q.pop(0))
                    cnt += 1

            def normalize_T(qb, m, wo_qsubs=()):
                acc = pa_t[(qb, m)]
                if m == 0:
                    st_t[qb] = skp.tile([P, 2, QW], F16, tag="st", name=f"st{qb}")
                recs = []
                for hh in range(2):
                    rec = rp.tile([P, 4, 1], F32, tag="r", name=f"rec{qb}_{m}_{hh}")
                    nc.vector.reciprocal(out=rec[:], in_=acc[hh][:, :, DH:DH + 1])
                    recs.append(rec)
                for qs in range(4):
                    an = anp.tile([P, 2, DH], F16, tag="a", name=f"an{qb}_{m}_{qs}")
                    for hh in range(2):
                        nc.vector.tensor_scalar_mul(
                            out=an[:, hh, :], in0=acc[hh][:, qs, 0:DH],
                            scalar1=recs[hh][:, qs, :])
                    pt = pp.tile([P, P], F16, tag="pp", name=f"pt{qb}_{m}_{qs}")
                    nc.tensor.transpose(
                        pt[:], an[:].rearrange("p a b -> p (a b)"), ident[:])
                    nc.vector.tensor_copy(
                        out=st_t[qb][:, m, qs * P:(qs + 1) * P], in_=pt[:])
                    if qs in wo_qsubs:
                        wo_proj(qb, (qs,), drain_act=True)

            def wo_proj(qb, qsubs=(0, 1, 2, 3), drain_act=False):
                st = st_t[qb]
                for qs in qsubs:
                    ysb = yp.tile([P, D], F16, tag="y", name=f"ysb{qb}_{qs}")
                    for nb in range(2):
                        yps = pp.tile([P, 512], F32, tag="pp", name=f"yps{qb}_{qs}_{nb}")
                        for ch in range(2):
                            nc.tensor.matmul(
                                yps[:], st[:, ch, qs * P:(qs + 1) * P],
                                wo_sb[:, ch, nb * 512:(nb + 1) * 512],
                                start=(ch == 0), stop=(ch == 1))
                        # tail: ACT is idle once the last exp is done; the two
                        # halves drain on ACT and DVE in parallel, and each
                        # half DMAs out as soon as it lands
                        if drain_act and nb == 0:
                            nc.scalar.copy(
                                out=ysb[:, nb * 512:(nb + 1) * 512], in_=yps[:])
                        else:
                            nc.vector.tensor_copy(
                                out=ysb[:, nb * 512:(nb + 1) * 512], in_=yps[:])
                        if drain_act:
                            nc.sync.dma_start(
                                out=y.ap()[(qb * 4 + qs) * P:(qb * 4 + qs + 1) * P,
                                           nb * 512:(nb + 1) * 512],
                                in_=ysb[:, nb * 512:(nb + 1) * 512])
                    if not drain_act:
                        nc.sync.dma_start(
                            out=y.ap()[(qb * 4 + qs) * P:(qb * 4 + qs + 1) * P, :],
                            in_=ysb)

            # ---- emission schedule ----
            # Principle: exp on ACT is the global bottleneck (~133us busy)
            # and PE busy is nearly equal, so every phase must interleave
            # scores/exp with just enough other PE work. The warm (ctx) phase
            # is PE-heavy, so BOTH qb0 units' scores/exp streams run there
            # while unit (0,1)'s PVs are deferred (PV accumulation order is
            # free) until its pa banks free up after normalize(0,0). Dummy
            # matmuls at t=0 ramp the PE out of its low p-state before real
            # data lands. Steady units keep PVs 2 blocks behind scores so a
            # waiting PV never stalls score issue in the in-order PE queue.
            dumW = consts.tile([P, P], F16)
            dumM = consts.tile([P, QW], F16)
            nc.gpsimd.memset(dumW[:], 0.0)
            nc.gpsimd.memset(dumM[:], 0.0)
            for d in range(12):
                dmp = pp.tile([P, QW], F32, tag="pp", name=f"dum{d}")
                nc.tensor.matmul(dmp[:], dumW[:], dumM[:], start=True, stop=True)
            load_w(wk_sb, wk_d, 0)
            load_w(wq_sb, wq_d, 0)
            xc0 = xcp.tile([P, 2, KC, QW], F8, tag="xc", name="xc0")
            xq0 = xqp.tile([P, 2, KC, QW], F8, tag="xq", name="xq0")
            xc_t[0], xq_t[0] = xc0, xq0
            src_c = xtc_d.ap()[0:P, :].rearrange("p (l c s) -> p l c s", l=2, s=QW)
            src_q = xtq_d.ap()[0:P, :].rearrange("p (l c s) -> p l c s", l=2, s=QW)
            # interleave xc0/xq0 half-DMAs so the first K and Q projection
            # halves ping-pong with the (serial) DMA pool
            nc.sync.dma_start(out=xc0[:, :, 0:4, :], in_=src_c[:, :, 0:4, :])
            nc.sync.dma_start(out=xq0[:, :, 0:4, :], in_=src_q[:, :, 0:4, :])
            nc.sync.dma_start(out=xc0[:, :, 4:8, :], in_=src_c[:, :, 4:8, :])
            nc.sync.dma_start(out=xq0[:, :, 4:8, :], in_=src_q[:, :, 4:8, :])
            load_w(wk_sb, wk_d, 1)
            load_w(wq_sb, wq_d, 1)
            nc.sync.dma_start(
                out=wv_sb,
                in_=wv_d.ap().rearrange("p (l c f) -> p l c f", l=2, f=HD_C))
            load_xc(1)
            k_half(0, 0, 0)
            q_half(0, 0, 0)
            k_half(0, 0, 1)
            q_half(0, 0, 1)
            se(0, 0, 0)
            k_proj(0, 1)
            se(0, 0, 1)
            qpath(0, (1,))
            se(0, 1, 0)
            v_proj(0)
            load_late_consts()
            load_xc(2)
            load_xc(3)
            # Warm loop: stream (0,0) leads, stream (0,1) lags 2 blocks so an
            # exp is always ready; each slot carries <=1 quarter-projection +
            # one V block between scores, so no PE quantum outlasts one exp
            # shadow. (0,1)'s PVs defer entirely (pa banks busy until
            # normalize(0,0)); (0,0)'s trail by 2.
            for i in range(2, NKB):
                if i == 4:
                    load_xq(1)
                if i < 14:
                    # group g's 4 quarter-projections run at slots 4g-2..4g+1
                    q4 = (i + 2) % 4
                    k_half((i + 2) // 4, q4 // 2, q4 % 2)
                if i in (8, 9):
                    q_half(1, 0, i - 8)
                se(0, 1, i - 1)
                v_proj(i - 1)
                se(0, 0, i)
                # third stream: unit (1,0)'s first blocks ride the warm tail
                if i >= 10:
                    se(1, 0, i - 10)
                flush_pv(0, 0, n=1, keep=2)
            v_proj(15)
            se(0, 1, 15)

            # unit (1,0) remainder: drain deferred qb0 PVs in its ACT shadow
            se(1, 0, 6)
            flush_pv(0, 0)
            normalize_T(0, 0)
            se(1, 0, 7)
            flush_pv(0, 1, n=5)
            se(1, 0, 8)
            flush_pv(0, 1, n=5)
            se(1, 0, 9)
            flush_pv(0, 1, n=6)
            normalize_T(0, 1)
            for i in range(10, NKB):
                if i in (10, 11):
                    q_half(1, 1, i - 10)
                se(1, 0, i)
                # (1,0)'s own PVs ride into unit (1,1)'s slack entirely

            for u in range(3, 2 * NQB):
                qb, m = u // 2, u % 2
                pqb, pm = (u - 1) // 2, (u - 1) % 2
                norm_done = False
                for i in range(NKB):
                    se(qb, m, i)
                    # drain the previous unit's (possibly large) PV debt a
                    # few blocks per slot; normalize it once empty, then keep
                    # this unit's own PVs two blocks behind its scores
                    if pvq.get((pqb, pm)):
                        flush_pv(pqb, pm, n=3)
                        if not pvq.get((pqb, pm)) and not norm_done:
                            normalize_T(pqb, pm)
                            norm_done = True
                    else:
                        if not norm_done:
                            normalize_T(pqb, pm)
                            norm_done = True
                        flush_pv(qb, m, n=1, keep=2)
                    if m == 0 and qb >= 2 and i in (8, 10, 12, 14):
                        wo_proj(qb - 1, ((i - 8) // 2,))
                    if m == 0 and i in (12, 13):
                        q_half(qb, 1, i - 12)
                    if u == 3 and i in (6, 8, 10, 12):
                        wo_proj(0, ((i - 6) // 2,))
                    if m == 1 and qb + 1 < NQB:
                        if i == 6:
                            load_xq(qb + 1)
                        if i in (8, 9):
                            q_half(qb + 1, 0, i - 8)
            flush_pv(NQB - 1, 1)
            normalize_T(NQB - 1, 1, wo_qsubs=(0, 1, 2, 3))

    nc.compile()
    return nc


def _get_nc():
    if "nc" not in _CACHE:
        _CACHE["nc"] = _build()
    return _CACHE["nc"]


def _hilo(a, axis):
    """Split scaled f32 array into fp8 hi + fp8 lo stacked on `axis`."""
    import ml_dtypes

    hi = a.astype(ml_dtypes.float8_e4m3fn)
    lo = (a - hi.astype(np.float32)).astype(ml_dtypes.float8_e4m3fn)
    return np.ascontiguousarray(np.stack([hi, lo], axis=axis))


def _make_in_maps(query, context, Wq, Wk, Wv, Wo):
    ident = np.eye(P, dtype=np.float16)
    in_maps = []
    for c in range(8):
        b, g = c // 4, c % 4
        csl = slice(g * HD_C, (g + 1) * HD_C)
        # xT [D, S] -> [qb, p, hilo, c, s] -> [(qb p), (l c s)], scaled by SX
        xq = _hilo(
            (SX * query[b].T).astype(np.float32)
            .reshape(KC, P, NQB, QW).transpose(2, 1, 0, 3), axis=2
        ).reshape(NQB * P, 2 * KC * QW)
        xc = _hilo(
            (SX * context[b].T).astype(np.float32)
            .reshape(KC, P, NG, QW).transpose(2, 1, 0, 3), axis=2
        ).reshape(NG * P, 2 * KC * QW)
        # Wq/Wk [D, 256] -> pair-major [m, p, hilo, c, f], scaled by SW
        wq = _hilo(
            (SW * Wq[:, csl]).astype(np.float32)
            .reshape(KC, P, NPAIR, P).transpose(2, 1, 0, 3), axis=2
        ).reshape(NPAIR * P, 2 * KC * P)
        wk = _hilo(
            (SW * Wk[:, csl]).astype(np.float32)
            .reshape(KC, P, NPAIR, P).transpose(2, 1, 0, 3), axis=2
        ).reshape(NPAIR * P, 2 * KC * P)
        # Wv [D, 256] -> [p, hilo, c, f], scaled by SW
        wv = _hilo(
            (SW * Wv[:, csl]).astype(np.float32)
            .reshape(KC, P, HD_C).transpose(1, 0, 2), axis=1
        ).reshape(P, 2 * KC * HD_C)
        # Wo rows [256, D] -> [p, (ch f)] with element [p, ch, f] = Wo[ch*128+p, f]
        wo = np.ascontiguousarray(
            Wo[csl, :].reshape(2, P, D).transpose(1, 0, 2)
            .reshape(P, 2 * D).astype(np.float16))
        in_maps.append({
            "xtq": xq,
            "xtc": xc,
            "wq": wq,
            "wk": wk,
            "wv": wv,
            "wo": wo,
            "identity": ident,
        })
    return in_maps


def kernel(query, context, Wq, Wk, Wv, Wo, bo):
    from concourse.bass_utils import run_bass_kernel_spmd

    query = np.asarray(query, dtype=np.float32)
    context = np.asarray(context, dtype=np.float32)
    Wq = np.asarray(Wq, dtype=np.float32)
    Wk = np.asarray(Wk, dtype=np.float32)
    Wv = np.asarray(Wv, dtype=np.float32)
    Wo = np.asarray(Wo, dtype=np.float32)
    bo = np.asarray(bo, dtype=np.float32)

    nc = _get_nc()
    in_maps = _make_in_maps(query, context, Wq, Wk, Wv, Wo)
    res = run_bass_kernel_spmd(nc, in_maps, core_ids=list(range(8)))
    out = np.zeros((B, S, D), np.float32)
    for c in range(8):
        out[c // 4] += np.asarray(res.results[c]["y"], dtype=np.float32)
    out += bo[None, None, :]
    return out
